# revision 1
# baseline (speedup 1.0000x reference)
"""Trainium2 Bass kernel for a backward-Euler 1D diffusion step (Thomas solve).

The tridiagonal system has constant coefficients (a=-r, b=1+2r, c=-r) except
at the two Dirichlet boundary rows.  The Thomas c' coefficient converges to a
fixed point p (|p| = beta < 1), turning both sweeps into constant-coefficient
first-order linear recurrences whose influence decays like beta^k.  With a
halo of W elements (beta^W ~ 1e-11) every chunk of the grid can be scanned
independently:

  F_i = d_i + beta * F_{i-1}      (forward,  d = raw rhs)
  G_i = F_i + beta * G_{i+1}      (backward)
  x_i = G_i / denom*              (denom* = fixed-point denominator)

Device: 8 cores x 128 partitions x 4096-element rows with +-W halos.
DVE tensor_tensor_scan does each sweep (backward via reversed access
patterns); the final 1/denom* scale is folded into the input on the host
(both sweeps are linear).  The exact (varying-coefficient) treatment near
the two boundaries is done on the host and patched in.
"""

import sys

if "/opt/trn_rl_repo" not in sys.path:
    sys.path.insert(0, "/opt/trn_rl_repo")

import numpy as np

import concourse.bass as bass
import concourse.mybir as mybir
from concourse.bass_utils import run_bass_kernel_spmd

F32 = np.float32

# Problem constants (from the nn.Module init args)
D_COEF = 1e-05
DX = 1e-04
NX = 4_194_304

NCORES = 8
P = 128                    # SBUF partitions
M = NX // NCORES           # elements per core
CB = M // P                # elements per partition row (owned)
assert CB * P * NCORES == NX


def _rev(ap):
    """Reverse an AP along its innermost (free) dimension."""
    a = ap.copy()
    pairs = [list(x) for x in a.ap]
    st, ct = pairs[-1]
    assert st == 1, f"can only reverse contiguous innermost dim, got step {st}"
    pairs[-1] = [-1, ct]
    return bass.AP(a.tensor, a.offset + (ct - 1), pairs)


def _params(dt):
    """fp32 scalar parameters mirroring the reference arithmetic."""
    dt = F32(dt)
    dx2 = F32(F32(DX) * F32(DX))
    r = F32(F32(F32(D_COEF) * dt) / dx2)
    b = F32(F32(1.0) + F32(2.0) * r)
    # fixed point of c'_{i} = -r / (b + r*c'_{i-1})  (c' starts at 0)
    cp = F32(0.0)
    for _ in range(20000):
        denom = F32(b - F32(F32(-r) * cp))
        cp_new = F32(F32(-r) / denom)
        if cp_new == cp:
            break
        cp = cp_new
    denom = F32(b - F32(F32(-r) * cp))
    beta = F32(F32(r) / denom)      # multiplier of both recurrences
    sc = F32(F32(1.0) / denom)      # final scale 1/denom*
    return r, b, float(beta), float(sc)


def _halo(beta):
    """Halo W: beta^W <~ 1e-8 (25x below fp32 noise), multiple of 64."""
    if beta < 1e-6:
        need = 64
    elif beta < 1.0:
        need = int(np.ceil(np.log(1e8) / -np.log(beta)))
    else:
        need = 1024
    need = min(max(need, 64), 1024)
    W = 64 * int(np.ceil(need / 64))
    return W, 640


_BUILD_CACHE = {}


def _tiles(a, b, tw, small_first=0, small_last=0):
    """Split [a,b) into tile (start,end) pairs of ~tw, optional small edges."""
    span = b - a
    ws = []
    if small_first and span > small_first:
        ws.append(small_first); span -= small_first
    last = small_last if (small_last and span > small_last) else 0
    span -= last
    nmid = max(1, round(span / tw))
    base = span // nmid
    ws += [base + (1 if i < span - base * nmid else 0) for i in range(nmid)]
    if last:
        ws.append(last)
    out, off = [], a
    for w in ws:
        out.append((off, off + w)); off += w
    assert off == b
    return out


def _build(beta, sc, W, TW, nseg=3, s_edge=768, s_first=1536):
    """Build the SPMD bass program for one core (all cores identical).

    One GLOBAL forward chain left-to-right over [0, R) (cross-segment
    chaining, no interior warm-ups).  The backward sweep is split into
    `nseg` independent segments [c_p, c_{p+1}+W) with a W warm-up each;
    segment p's backward chain interleaves with the forward tiles of
    segment p+1, so finished output streams out while later input still
    loads.  The rightmost (last-processed) segment is smallest to cut the
    output-DMA drain after the final scan.
    """
    key = (beta, sc, W, TW, nseg, s_edge, s_first)
    if key in _BUILD_CACHE:
        return _BUILD_CACHE[key]

    R = CB + 2 * W
    # segment cuts c_0=W < ... < c_nseg = W+CB ; rightmost span smallest
    ov = getattr(_build, "_spans", None)
    if ov is None and nseg == 3 and s_edge == 768 and s_first == 1536:
        # tuned asymmetric profile (cost-model swept): decreasing spans give
        # each later segment's backward sweep progressively earlier starts
        ov = (1440, 1056, 854, 746)
    if ov:
        assert sum(ov) == CB
        nseg = len(ov)
    sl_ = min(s_edge, max(CB // (2 * nseg), 256))
    rest = CB - sl_
    if ov:
        spans = list(ov)
    elif nseg == 1:
        spans = [CB]
    elif s_first:
        sf = min(s_first, rest - 256)
        mid = rest - sf
        spans = [sf] + [mid // (nseg - 2) + (1 if i < mid % (nseg - 2) else 0)
                        for i in range(nseg - 2)] + [sl_] if nseg > 2 else [sf + mid, sl_]
    else:
        spans = [rest // (nseg - 1) + (1 if i < rest % (nseg - 1) else 0)
                 for i in range(nseg - 1)] + [sl_]
    cuts = [W]
    for s in spans:
        cuts.append(cuts[-1] + s)
    assert cuts[-1] == W + CB

    # forward tiles: global tiling of [0, R) with forced edges at cuts;
    # tiny first tile for a fast pipeline start
    fwd_tiles = []
    for p in range(nseg):
        lo = 0 if p == 0 else cuts[p]
        hi = R if p == nseg - 1 else cuts[p + 1]
        if p == 0:
            # ramped early tiles: DVE tracks the arriving DMA stream closely
            ws, rem = [], hi - lo
            ramp = getattr(_build, "_ramp", None) or (W + 64, 416, 448, 512)
            for w in ramp:
                if rem - w < TW // 2:
                    break
                ws.append(w); rem -= w
            ts_ = _tiles(lo + sum(ws), hi, TW) if rem else []
            off = lo
            tl = []
            for w in ws:
                tl.append((off, off + w)); off += w
            fwd_tiles.append(tl + ts_)
        else:
            # tiny LAST forward tile: it gates the final backward tiles
            # (full coverage), so finishing it quickly after the last
            # input arrives pulls in the whole end chain
            fl = getattr(_build, "_flast", 192) if p == nseg - 1 else 0
            fwd_tiles.append(_tiles(lo, hi, TW, small_last=fl))
    # backward tiles: segment p covers [c_p, c_{p+1}+W), rightmost W is
    # warm-up; last-processed segment ends in a small tile (small out tail)
    bwd_tiles = []
    bsmall = getattr(_build, "_bsmall", None)
    for p in range(nseg):
        blo, bhi = cuts[p], min(cuts[p + 1] + W, R)
        sf_ = (W + 128) if p == nseg - 1 else (bsmall or 0)
        bwd_tiles.append(_tiles(blo, bhi, TW, small_first=sf_))

    nc = bass.Bass(trn_type="TRN2")
    cin = nc.dram_tensor("cin", [M + 2 * W], mybir.dt.float32, kind="ExternalInput")
    xout = nc.dram_tensor("xout", [M], mybir.dt.float32, kind="ExternalOutput")

    # ---- DVE schedule: entries ("f"/"b", p, (t0,t1)) ----
    # Coverage-driven merge: a backward tile is eligible only once the
    # forward chain has covered its full read range [t0, t1) -- with W
    # larger than a forward tile this can span several forward tiles, so
    # a fixed zip would order reads before their producers (race).
    fqueue = [("f", p, t) for p in range(nseg) for t in fwd_tiles[p]]
    bqueue = [("b", p, t) for p in range(nseg) for t in reversed(bwd_tiles[p])]
    sched = []
    cov = 0
    fi = bi = 0
    bquota = getattr(_build, "_bquota", 1)
    while fi < len(fqueue) or bi < len(bqueue):
        # emit up to `bquota` ready backward tiles per forward tile: the
        # DVE drains backward work during DMA-paced stretches without
        # starving the forward chain (which gates later coverage)
        q = 0
        while bi < len(bqueue) and bqueue[bi][2][1] <= cov and \
                (q < bquota or fi >= len(fqueue)):
            sched.append(bqueue[bi]); bi += 1; q += 1
        if fi < len(fqueue):
            sched.append(fqueue[fi]); cov = fqueue[fi][2][1]; fi += 1
        elif bi >= len(bqueue):
            break
        else:
            assert bqueue[bi][2][1] <= cov, "backward tile never covered"
    scan_idx = {e: i + 1 for i, e in enumerate(sched)}
    all_f = [e for e in sched if e[0] == "f"]

    # build-time invariants (host side, zero runtime cost):
    # every backward tile must follow all forward tiles covering its range
    for i, e in enumerate(sched):
        if e[0] == "b":
            t0, t1 = e[2]
            for x in all_f:
                if x[2][0] < t1 and x[2][1] > t0:
                    assert scan_idx[x] < scan_idx[e], (e, x)
    # forward chain contiguity
    fts_all = [t for k, _, t in sched if k == "f"]
    assert fts_all[0][0] == 0 and fts_all[-1][1] == R
    for a_, b_ in zip(fts_all, fts_all[1:]):
        assert a_[1] == b_[0], (a_, b_)
    # backward tiles cover each segment's [c_p, c_{p+1}+W) contiguously
    for p in range(nseg):
        bt = bwd_tiles[p]
        assert bt[0][0] == cuts[p] and bt[-1][1] == min(cuts[p + 1] + W, R)
        for a_, b_ in zip(bt, bt[1:]):
            assert a_[1] == b_[0]

    in_order = [t for p in range(nseg) for t in fwd_tiles[p]]

    from contextlib import ExitStack
    with ExitStack() as stack:
        tin = stack.enter_context(nc.sbuf_tensor("tin", [P, R], mybir.dt.float32))
        tf = stack.enter_context(nc.sbuf_tensor("tf", [P, R], mybir.dt.float32))
        tbe = stack.enter_context(nc.sbuf_tensor("tbe", [P, 1], mybir.dt.float32))

        def bcast(w):
            return bass.AP(tbe[:].tensor, 0, [[1, P], [0, w]])
        tgs = [stack.enter_context(
                   nc.sbuf_tensor(f"tg{p}",
                                  [P, bwd_tiles[p][-1][1] - bwd_tiles[p][0][0]],
                                  mybir.dt.float32))
               for p in range(nseg)]
        g0 = [bwd_tiles[p][0][0] for p in range(nseg)]
        in_sems = {t: stack.enter_context(nc.semaphore(f"in{t[0]}"))
                   for t in in_order}
        dve_sem = stack.enter_context(nc.semaphore("dve_sem"))
        dma_out_sem = stack.enter_context(nc.semaphore("dma_out_sem"))
        block = stack.enter_context(nc.Block())

        # out-DMA list in scan-completion order
        outs = []
        for e in sched:
            kind, p, (t0, t1) = e
            if kind != "b":
                continue
            a0, a1 = max(t0, cuts[p]), min(t1, cuts[p + 1])
            if a0 < a1:
                outs.append((scan_idx[e], p, a0, a1))

        @block.sync
        def _(sync):
            for t in in_order:
                src = bass.AP(cin, t[0], [[CB, P], [1, t[1] - t[0]]])
                sync.dma_start(tin[:, t[0]:t[1]], src).then_inc(in_sems[t], 16)
            for (si, p, a0, a1) in outs:
                sync.wait_ge(dve_sem, si)
                dst = bass.AP(xout, a0 - W, [[CB, P], [1, a1 - a0]])
                sync.dma_start(dst, tgs[p][:, a0 - g0[p]:a1 - g0[p]]).then_inc(
                    dma_out_sem, 16)
            # REQUIRED: without this wait the kernel can signal completion
            # while output DMAs are still in flight -- empirically corrupts
            # outputs nondeterministically (seen at W=640 tilings).
            sync.wait_ge(dma_out_sem, 16 * len(outs))

        @block.vector
        def _(vector):
            vector.memset(tbe[:], beta)
            for e in sched:
                kind, p, (t0, t1) = e
                w = t1 - t0
                if kind == "f":
                    vector.wait_ge(in_sems[(t0, t1)], 16)
                    # global chain across segments
                    pe = next((x for x in all_f if x[2][1] == t0), None)
                    if pe:
                        vector.wait_ge(dve_sem, scan_idx[pe])
                    init = 0.0 if pe is None else tf[:, t0 - 1:t0]
                    vector.tensor_tensor_scan(
                        tf[:, t0:t1], bcast(w), tin[:, t0:t1], init,
                        op0=mybir.AluOpType.mult, op1=mybir.AluOpType.add,
                    ).then_inc(dve_sem, 1)
                else:
                    pe = next((x for x in sched
                               if x[0] == "b" and x[1] == p and x[2][0] == t1),
                              None)
                    # all earlier-scheduled producers of this tf range must
                    # have DRAINED (stream reads race with the DVE pipe)
                    need = scan_idx[pe] if pe else 0
                    for x in all_f:
                        if scan_idx[x] < scan_idx[e] and                                 x[2][0] < t1 and x[2][1] > t0:
                            need = max(need, scan_idx[x])
                    if need:
                        vector.wait_ge(dve_sem, need)
                    g = tgs[p]
                    init = (0.0 if pe is None
                            else g[:, t1 - g0[p]:t1 - g0[p] + 1])
                    vector.tensor_tensor_scan(
                        _rev(g[:, t0 - g0[p]:t1 - g0[p]]), bcast(w),
                        _rev(tf[:, t0:t1]), init,
                        op0=mybir.AluOpType.mult, op1=mybir.AluOpType.add,
                    ).then_inc(dve_sem, 1)

    _BUILD_CACHE[key] = nc
    return nc


def _host_patches(C, dt, C_surf, C_bulk, r, b, beta, sc, W, x_dev):
    """Exact fp32 Thomas near both boundaries; returns (left, right) patches."""
    n = C.shape[0]
    K1 = 4 * W                 # left exact region
    Wp = 2 * W                 # right patch length

    # ---- left: exact forward coefficients from i=0 ----
    cp = np.empty(K1, np.float32)
    dp = np.empty(K1, np.float32)
    a_i = F32(-r)
    cp[0] = F32(0.0)
    dp[0] = F32(C_surf)
    for i in range(1, K1):
        denom = F32(b - F32(a_i * cp[i - 1]))
        cp[i] = F32(F32(-r) / denom)
        dp[i] = F32(F32(C[i] - F32(a_i * dp[i - 1])) / denom)
    left = np.empty(K1, np.float32)
    xn = F32(x_dev[K1])        # device value just right of the exact region
    for i in range(K1 - 1, -1, -1):
        xn = F32(dp[i] - F32(cp[i] * xn))
        left[i] = xn

    # ---- right: d' via warm-up scan, then exact backward from x_{n-1} ----
    j0 = n - 1 - Wp - 2 * W
    dpr = np.empty(n - 1 - j0, np.float32)   # d' for j0 .. n-2
    s = F32(0.0)
    rbeta = F32(beta)
    rsc = F32(sc)
    for idx, jj in enumerate(range(j0, n - 1)):
        s = F32(F32(F32(C[jj]) * rsc) + F32(rbeta * s))
        dpr[idx] = s
    right = np.empty(Wp + 1, np.float32)
    xn = F32(C_bulk)
    right[Wp] = xn
    for k in range(Wp - 1, -1, -1):
        jj = n - 1 - Wp + k
        xn = F32(dpr[jj - j0] + F32(rbeta * xn))
        right[k] = xn
    return K1, left, Wp, right


def kernel(C, dt, C_surf, C_bulk):
    C = np.ascontiguousarray(np.asarray(C, dtype=np.float32))
    n = C.shape[0]
    assert n == NX, f"kernel hardcoded for {NX}, got {n}"

    r, b, beta, sc = _params(np.float32(np.asarray(dt)))
    W, TW = _halo(beta)
    nc = _build(beta, sc, W, TW)

    # final 1/denom* scale folded into the input (both sweeps are linear)
    cpad = np.zeros(n + 2 * W, np.float32)
    np.multiply(C, F32(sc), out=cpad[W:W + n], dtype=np.float32)
    in_maps = [
        {"cin": np.ascontiguousarray(cpad[k * M:k * M + M + 2 * W])}
        for k in range(NCORES)
    ]
    res = run_bass_kernel_spmd(nc, in_maps, core_ids=list(range(NCORES)))
    x = np.concatenate([res.results[k]["xout"] for k in range(NCORES)])

    K1, left, Wp, right = _host_patches(
        C, dt, np.float32(np.asarray(C_surf)), np.float32(np.asarray(C_bulk)),
        r, b, beta, sc, W, x)
    x[:K1] = left
    x[n - 1 - Wp:] = right
    return x

